# revision 11
# baseline (speedup 1.0000x reference)
"""Trainium2 Bass kernel for nn_BitwiseTasNet (encoder + 32 linear residual
blocks + sigmoid mask + transposed-conv decoder).

v5 "collapsed-D" restructuring: the residual block has NO nonlinearity
(bn1 -> 1x1(C->D) -> bn2 -> depthwise 3-tap -> bn3 -> 1x1(D->C)), so the
whole block is linear in h and collapses host-side to three [C,C] matrices
applied to dilation-shifted h:

    r(t) = M0 @ h(t-d) + M1 @ h(t) + M2 @ h(t+d) + const
    Mk   = W2 . diag(a3*wd_k) . diag(a2) . W1 . diag(a1)

This removes the D=512 intermediate entirely: no GEMM1 PSUM evictions, no
depthwise-tap vector ops.  Per block the device work is 24 [128x128]
matmuls on zero-haloed bf16 h tiles + 4 rank-1 edge-correction matmuls
(the dconv zero-pads its *input*, so at the d-wide edges the folded
constants deviate by a per-block vector; exact fix via K=1 matmuls of
e (x) ones).  The residual stream h stays fp32 in PSUM across all blocks
(GEMM accumulates with start=False); only the bf16 working copy hb is
evicted per block, split into 3 column pieces so the next block's chunk-A
matmuls start as soon as [0:640) is present.  Uniform constants propagate
through the linear blocks host-side (s-recursion) and land in the final
sigmoid bias.  Validated 5.6e-3 rel_l2 vs reference in fp64/bf16 numpy.

Sharding: data-parallel over batch N=4 on 4 cores (pair-collectives ~20us
per shot on this stack - cross-core comm per block is not viable).
"""
import sys
import numpy as np
import ml_dtypes

sys.path.insert(0, "/opt/trn_rl_repo")

from concourse import bass, bacc, tile, mybir  # noqa: E402
from concourse.bass_utils import run_bass_kernel_spmd  # noqa: E402

# model dims (hardcoded per contract)
N, CIN, T = 4, 1, 8000
C, D, K = 256, 512, 3
FK, FS = 20, 10
REPEATS, BLOCKS = 4, 8
NB = REPEATS * BLOCKS
EPS = 1e-5
L = 803
PAD = 128              # h-tile halo (max dilation)
TW = PAD + L + PAD
CHUNKS = [(0, 512), (512, L)]   # psum-bank-aligned matmul free-dim chunks
B1E = 640              # hb eviction piece boundary: [512:640) unblocks chunk A

F32 = mybir.dt.float32
BF16 = mybir.dt.bfloat16
bf16 = ml_dtypes.bfloat16
AF = mybir.ActivationFunctionType


# ----------------------------------------------------------------- host math
def fold_params(inp):
    p = {k: np.asarray(v, dtype=np.float64) for k, v in inp.items()}
    a = {}
    for nm in ('bn1', 'bn2', 'bn3'):
        sc = p[nm + '_g'] / np.sqrt(p[nm + '_v'] + EPS)
        sh = p[nm + '_b'] - p[nm + '_m'] * sc
        a[nm] = (sc, sh)
    a1, c1 = a['bn1']; a2, c2 = a['bn2']; a3, c3 = a['bn3']
    W1 = p['w1'][:, :, :, 0]    # [NB, D, C]
    W2 = p['w2'][:, :, :, 0]    # [NB, C, D]
    wd = p['wd'][:, :, 0, :]    # [NB, D, 3]

    # bn2 o 1x1 o bn1 as affine: x2 = A h + beta
    A = a2[:, :, None] * W1 * a1[:, None, :]                  # [NB, D, C]
    beta = a2 * np.einsum('idc,ic->id', W1, c1) + c2          # [NB, D]
    M = np.zeros((NB, 3, C, C)); q = np.zeros((NB, 3, C))
    for k in range(3):
        g = a3 * wd[:, :, k]                                  # [NB, D]
        M[:, k] = np.einsum('icd,id,idx->icx', W2, g, A)
        q[:, k] = np.einsum('icd,id,id->ic', W2, g, beta)
    qc3 = np.einsum('icd,id->ic', W2, c3)                     # [NB, C]

    # uniform-constant recursion + per-block edge deviation vectors
    s = np.zeros((NB + 1, C))
    e0 = np.zeros((NB, C)); e2 = np.zeros((NB, C))
    for i in range(NB):
        Msum = M[i, 0] + M[i, 1] + M[i, 2]
        s[i + 1] = s[i] + Msum @ s[i] + q[i].sum(0) + qc3[i]
        e0[i] = -(M[i, 0] @ s[i] + q[i, 0])
        e2[i] = -(M[i, 2] @ s[i] + q[i, 2])
    return dict(M=M, e0=e0, e2=e2, sig_bias=s[NB],
                Wenc=p['w_enc'][:, 0, :], Wdec=p['w_dec'][:, 0, :])


def im2col(x):
    xp = np.zeros((N, T + 2 * FK), dtype=np.float32)
    xp[:, FK:FK + T] = np.asarray(x, np.float32)[:, 0, :]
    idx = FS * np.arange(L)[None, :] + np.arange(FK)[:, None]  # [FK, L]
    return xp[:, idx]                                          # [N, FK, L]


def pack_host(f):
    """Pack folded params into DMA-friendly arrays."""
    # mt[i, :, ((k*2+kk)*2+mc)*128:+128] = M[i,k][mc-slice, kk-slice].T
    mt = np.zeros((NB, 128, 12 * 128), np.float32)
    for k in range(3):
        for kk in range(2):
            for mc in range(2):
                off = ((k * 2 + kk) * 2 + mc) * 128
                mt[:, :, off:off + 128] = np.transpose(
                    f['M'][:, k, mc * 128:(mc + 1) * 128,
                           kk * 128:(kk + 1) * 128], (0, 2, 1))
    # edge vectors as K=1 lhsT rows: ed[0, (i*4+side*2+mc)*128:+128]
    ed = np.zeros((1, NB * 4 * 128), np.float32)
    for i in range(NB):
        for side, e in ((0, f['e0']), (1, f['e2'])):
            for mc in range(2):
                off = (i * 4 + side * 2 + mc) * 128
                ed[0, off:off + 128] = e[i][mc * 128:(mc + 1) * 128]
    vecs = np.zeros((128, 2), np.float32)
    for mc in range(2):
        vecs[:, mc] = f['sig_bias'][mc * 128:(mc + 1) * 128]
    wenct = f['Wenc'].T.astype(np.float32)                     # [20, 256]
    wdect = np.zeros((128, 40), np.float32)
    for k in range(2):
        wdect[:, k * 20:(k + 1) * 20] = f['Wdec'][k * 128:(k + 1) * 128, :]
    return dict(mt=mt.astype(bf16), ed=ed.astype(bf16),
                wenct=wenct.astype(bf16), wdect=wdect.astype(bf16), vecs=vecs)


# -------------------------------------------------------------- device build
def build_nc(n_cores=4, n_blocks=NB):
    nc = bacc.Bacc("TRN2", target_bir_lowering=False, debug=False,
                   num_devices=n_cores)
    xcol_d = nc.dram_tensor("xcol", [FK, L], BF16, kind="ExternalInput")
    mt_d = nc.dram_tensor("mt", [NB, 128, 12 * 128], BF16,
                          kind="ExternalInput")
    ed_d = nc.dram_tensor("ed", [1, NB * 4 * 128], BF16, kind="ExternalInput")
    wenc_d = nc.dram_tensor("wenct", [FK, C], BF16, kind="ExternalInput")
    wdec_d = nc.dram_tensor("wdect", [128, 40], BF16, kind="ExternalInput")
    vecs_d = nc.dram_tensor("vecs", [128, 2], F32, kind="ExternalInput")
    out_d = nc.dram_tensor("out", [10, 800], F32, kind="ExternalOutput")

    with tile.TileContext(nc) as tc:
        with (
            tc.tile_pool(name="fix", bufs=1) as fix,
            tc.tile_pool(name="mp", bufs=6) as mpool,
            tc.tile_pool(name="hps", bufs=1, space="PSUM") as hps,
            tc.tile_pool(name="pd", bufs=2, space="PSUM") as pdp,
        ):
            vecs = fix.tile([128, 2], F32, tag="vecs")
            xcol = fix.tile([FK, L], BF16, tag="xcol")
            wenc = fix.tile([FK, C], BF16, tag="wenc")
            wdec = fix.tile([128, 40], BF16, tag="wdec")
            ed = fix.tile([1, NB * 4 * 128], BF16, tag="ed")
            ones = fix.tile([1, 128], BF16, tag="ones")
            hb = [[fix.tile([128, TW], BF16, tag=f"hb{g}{m}",
                           name=f"hb{g}{m}") for m in range(2)]
                  for g in range(2)]
            xe = [fix.tile([128, L], BF16, tag=f"xe{m}", name=f"xe{m}")
                  for m in range(2)]
            msk = [fix.tile([128, L], BF16, tag=f"mk{m}", name=f"mk{m}")
                   for m in range(2)]
            yy = [fix.tile([128, L], BF16, tag=f"y{m}", name=f"y{m}")
                  for m in range(2)]
            outsb = fix.tile([10, 800], F32, tag="outsb")
            # persistent residual stream in PSUM, one tile per bank so
            # chunk-A eviction reads never false-depend on chunk-B matmuls
            hpA = [hps.tile([128, 512], F32, tag=f"hpA{m}", name=f"hpA{m}")
                   for m in range(2)]
            hpB = [hps.tile([128, L - 512], F32, tag=f"hpB{m}",
                            name=f"hpB{m}") for m in range(2)]

            nc.sync.dma_start(out=vecs[:], in_=vecs_d.ap())
            nc.sync.dma_start(out=xcol[:], in_=xcol_d.ap())
            nc.sync.dma_start(out=wenc[:], in_=wenc_d.ap())
            nc.sync.dma_start(out=wdec[:], in_=wdec_d.ap())
            nc.sync.dma_start(out=ed[:], in_=ed_d.ap())
            nc.gpsimd.memset(ones[:], 1.0)

            # zero halos once (never written again)
            for g in range(2):
                for t in hb[g]:
                    nc.gpsimd.memset(t[:, 0:PAD], 0.0)
                    nc.gpsimd.memset(t[:, PAD + L:TW], 0.0)
            # prefetch the sigmoid ACT table during block 0
            nc.scalar.activation(outsb[:, 0:2], outsb[:, 0:2], AF.Sigmoid)

            # ---- encoder: hp = Wenc @ xcol (opens the h accumulation) ----
            for mc in range(2):
                nc.tensor.matmul(
                    hpA[mc][:], wenc[:, mc * 128:(mc + 1) * 128],
                    xcol[:, 0:512], start=True, stop=False)
                nc.tensor.matmul(
                    hpB[mc][:], wenc[:, mc * 128:(mc + 1) * 128],
                    xcol[:, 512:L], start=True, stop=False)
                nc.scalar.copy(hb[0][mc][:, PAD:PAD + 512], hpA[mc][:])
                nc.scalar.copy(hb[0][mc][:, PAD + 512:PAD + L], hpB[mc][:])
                nc.vector.tensor_copy(xe[mc][:, 0:512], hpA[mc][:])
                nc.vector.tensor_copy(xe[mc][:, 512:L], hpB[mc][:])

            # ---- residual blocks ----
            for i in range(n_blocks):
                d = 2 ** (i % BLOCKS)
                last = (i == n_blocks - 1)
                hbg = hb[i % 2]
                hbn = hb[(i + 1) % 2]
                mtl = mpool.tile([128, 12 * 128], BF16, tag="mt", name="mt")
                nc.sync.dma_start(out=mtl[:], in_=mt_d.ap()[i])

                # left-edge corrections first: independent PE work that
                # fills the eviction-handoff gap at the block boundary
                for mc in range(2):
                    eo = (i * 4 + 0 * 2 + mc) * 128
                    nc.tensor.matmul(
                        hpA[mc][:, 0:d], ed[:, eo:eo + 128],
                        ones[:, 0:d], start=False, stop=False)
                # chunk A
                for k in range(3):
                    sh = (k - 1) * d
                    for kk in range(2):
                        for mc in range(2):
                            off = ((k * 2 + kk) * 2 + mc) * 128
                            nc.tensor.matmul(
                                hpA[mc][:],
                                mtl[:, off:off + 128],
                                hbg[kk][:, PAD + sh:PAD + 512 + sh],
                                start=False, stop=(last and k == 2 and kk == 1))
                if not last:
                    nc.scalar.copy(hbn[0][:, PAD:PAD + 512], hpA[0][:])
                    nc.vector.tensor_copy(hbn[1][:, PAD:PAD + 512], hpA[1][:])
                # chunk B
                for k in range(3):
                    sh = (k - 1) * d
                    for kk in range(2):
                        for mc in range(2):
                            off = ((k * 2 + kk) * 2 + mc) * 128
                            nc.tensor.matmul(
                                hpB[mc][:],
                                mtl[:, off:off + 128],
                                hbg[kk][:, PAD + 512 + sh:PAD + L + sh],
                                start=False, stop=False)
                # right-edge corrections (cols [L-d, L) = hpB[:, 291-d:291])
                for mc in range(2):
                    eo = (i * 4 + 1 * 2 + mc) * 128
                    nc.tensor.matmul(
                        hpB[mc][:, L - 512 - d:L - 512], ed[:, eo:eo + 128],
                        ones[:, 0:d], start=False, stop=(last and mc == 1))
                if not last:
                    nc.scalar.copy(hbn[0][:, PAD + 512:PAD + B1E],
                                   hpB[0][:, 0:B1E - 512])
                    nc.vector.tensor_copy(hbn[1][:, PAD + 512:PAD + B1E],
                                          hpB[1][:, 0:B1E - 512])
                    nc.scalar.copy(hbn[0][:, PAD + B1E:PAD + L],
                                   hpB[0][:, B1E - 512:L - 512])
                    nc.vector.tensor_copy(hbn[1][:, PAD + B1E:PAD + L],
                                          hpB[1][:, B1E - 512:L - 512])

            # ---- mask + decoder ----
            # sigmoid+mask per chunk so decoder chunk A (needs yy cols
            # [1,512) only) overlaps the chunk-B sigmoid
            for mc in range(2):
                nc.scalar.activation(msk[mc][:, 0:512], hpA[mc][:], AF.Sigmoid,
                                     bias=vecs[:, mc:mc + 1])
                nc.vector.tensor_mul(yy[mc][:, 0:512], xe[mc][:, 0:512],
                                     msk[mc][:, 0:512])
            for mc in range(2):
                nc.scalar.activation(msk[mc][:, 512:L], hpB[mc][:], AF.Sigmoid,
                                     bias=vecs[:, mc:mc + 1])
                nc.vector.tensor_mul(yy[mc][:, 512:L], xe[mc][:, 512:L],
                                     msk[mc][:, 512:L])

            for (c0, c1) in [(0, 510), (510, 800)]:
                po = pdp.tile([10, 512], F32, tag="pd", name="po")
                for k in range(2):
                    nc.tensor.matmul(
                        po[:, 0:c1 - c0], wdec[:, k * 20:k * 20 + 10],
                        yy[k][:, c0 + 2:c1 + 2], start=(k == 0), stop=False)
                    nc.tensor.matmul(
                        po[:, 0:c1 - c0], wdec[:, k * 20 + 10:k * 20 + 20],
                        yy[k][:, c0 + 1:c1 + 1], start=False, stop=(k == 1))
                nc.scalar.copy(outsb[:, c0:c1], po[:, 0:c1 - c0])
            nc.sync.dma_start(out=out_d.ap(), in_=outsb[:])

    nc.compile()
    return nc


# ------------------------------------------------------------------- driver
_CACHE = {}


def _get_nc(n_cores, n_blocks):
    key = (n_cores, n_blocks)
    if key not in _CACHE:
        _CACHE[key] = build_nc(n_cores, n_blocks)
    return _CACHE[key]


def run(inputs, n_blocks=NB, trace=False):
    f = fold_params(inputs)
    pk = pack_host(f)
    xc = im2col(inputs['x']).astype(bf16)
    n_cores = 4
    nc = _get_nc(n_cores, n_blocks)
    in_maps = []
    for n in range(n_cores):
        in_maps.append(dict(
            xcol=xc[n], mt=pk['mt'], ed=pk['ed'],
            wenct=pk['wenct'], wdect=pk['wdect'], vecs=pk['vecs']))
    res = run_bass_kernel_spmd(nc, in_maps, list(range(n_cores)), trace=trace)
    out = np.zeros((N, CIN, T), np.float32)
    for n in range(n_cores):
        out[n, 0, :] = res.results[n]['out'].T.reshape(T)
    return out, res


def kernel(**inputs):
    out, _ = run(inputs)
    return out
